# revision 22
# baseline (speedup 1.0000x reference)
"""Trainium2 Bass kernel: GQA causal sliding-window self-attention.

Sharding: 8 cores = DP2 (batch) x TP4 (head groups). Core c: b=c//4, tp=c%4.
Each core: 4 q-heads, 2 kv-heads, wproj input-slice; host sums 4 TP partials.

v2: single act-table regime (sigmoid+exp only), DVE fast-rsqrt rmsnorm,
PE-matmul key shift, t-sliced streaming x loads, fused square+reduce,
3-op rope, unified PSUM pools, bf16 output.
"""
import sys

sys.path.insert(0, "/opt/trn_rl_repo")

import numpy as np
import ml_dtypes

import concourse.bass as bass
import concourse.mybir as mybir
import concourse.tile as tile
from concourse import bacc
from concourse.bass_utils import run_bass_kernel_spmd
from concourse.masks import make_identity

bf16 = ml_dtypes.bfloat16
FP32 = mybir.dt.float32
BF16 = mybir.dt.bfloat16
I32 = mybir.dt.int32
T = 2048
NT = 16          # t tiles of 128
NCC = 16         # contraction chunks of 128 over C=2048
EPS = float(np.finfo(np.float32).eps)
MAGIC = 0x5F3759DF
AF = mybir.ActivationFunctionType
ALU = mybir.AluOpType
AX = mybir.AxisListType

_CACHE = {}


def _swap_halves(t):
    """AP over tile [128, H, 128] presenting [128, H, 2, 64] with the two
    64-halves of the last dim exchanged (via negative stride)."""
    ap = t[:, :, :]
    p = list(ap.ap[0])
    h = list(ap.ap[1])
    return bass.AP(ap.tensor, ap.offset + 64, [p, h, [-64, 2], [1, 64]])


def _build_nc():
    nc = bacc.Bacc(None, target_bir_lowering=False)

    xTt = nc.dram_tensor("xTt", [NT, 128, NCC, 128], BF16, kind="ExternalInput")
    ve2 = nc.dram_tensor("ve2", [2048, 256], BF16, kind="ExternalInput")
    wqkv = nc.dram_tensor("wqkv", [2048, 1024], BF16, kind="ExternalInput")
    wp = nc.dram_tensor("wp", [512, 2048], BF16, kind="ExternalInput")
    wveg = nc.dram_tensor("wveg", [32, 2], BF16, kind="ExternalInput")
    wag = nc.dram_tensor("wag", [12, 4], BF16, kind="ExternalInput")
    cosd = nc.dram_tensor("cosd", [128, NT, 128], BF16, kind="ExternalInput")
    sind = nc.dram_tensor("sind", [128, NT, 128], BF16, kind="ExternalInput")
    mdiag = nc.dram_tensor("mdiag", [128, 128], BF16, kind="ExternalInput")
    mfar = nc.dram_tensor("mfar", [128, 128], BF16, kind="ExternalInput")
    s1t = nc.dram_tensor("s1t", [128, 128], BF16, kind="ExternalInput")
    s0t = nc.dram_tensor("s0t", [128, 128], BF16, kind="ExternalInput")
    s00t = nc.dram_tensor("s00t", [128, 128], BF16, kind="ExternalInput")
    out = nc.dram_tensor("out", [2048, 2048], BF16, kind="ExternalOutput")

    with tile.TileContext(nc) as tc:
        with (
            tc.tile_pool(name="big", bufs=1) as big,
            tc.tile_pool(name="work", bufs=2) as work,
            tc.tile_pool(name="kwork", bufs=3) as kwork,
            tc.tile_pool(name="exw", bufs=6) as exw,
            tc.tile_pool(name="small", bufs=4) as small,
            tc.tile_pool(name="ps", bufs=2, space="PSUM") as ps,
            tc.tile_pool(name="pst", bufs=4, space="PSUM") as pst,
            tc.tile_pool(name="pat", bufs=2, space="PSUM") as pat,
        ):
            # ---- resident inputs (ordered by DMA-arrival need-time) ----
            xT_sb = big.tile([128, NT, NCC, 128], BF16)
            nc.sync.dma_start(out=xT_sb[:, 0, 0:1, :], in_=xTt[0, :, 0:1, :])
            wveg_sb = big.tile([32, 2], BF16)
            nc.sync.dma_start(out=wveg_sb, in_=wveg[:, :])
            wag_sb = big.tile([12, 4], BF16)
            nc.sync.dma_start(out=wag_sb, in_=wag[:, :])
            wqkv_sb = big.tile([128, NCC, 1024], BF16)
            nc.sync.dma_start(out=wqkv_sb[:, 0, :], in_=wqkv[0:128, :])
            nc.sync.dma_start(out=xT_sb[:, 0, 1:NCC, :], in_=xTt[0, :, 1:NCC, :])
            for g in range(1, NCC):
                nc.sync.dma_start(
                    out=wqkv_sb[:, g, :],
                    in_=wqkv[128 * g : 128 * (g + 1), :],
                )
            nc.sync.dma_start(out=xT_sb[:, 1, :, :], in_=xTt[1])
            ve_sb = big.tile([128, NT, 256], BF16)
            nc.sync.dma_start(out=ve_sb, in_=ve2.rearrange("(i p) d -> p i d", p=128))
            cos_sb = big.tile([128, NT, 128], BF16)
            nc.sync.dma_start(out=cos_sb, in_=cosd[:, :, :])
            sin_sb = big.tile([128, NT, 128], BF16)
            nc.sync.dma_start(out=sin_sb, in_=sind[:, :, :])
            s1t_sb = big.tile([128, 128], BF16)
            nc.sync.dma_start(out=s1t_sb, in_=s1t[:, :])
            s0t_sb = big.tile([128, 128], BF16)
            nc.sync.dma_start(out=s0t_sb, in_=s0t[:, :])
            s00t_sb = big.tile([128, 128], BF16)
            nc.sync.dma_start(out=s00t_sb, in_=s00t[:, :])
            nc.sync.dma_start(out=xT_sb[:, 2, :, :], in_=xTt[2])
            mdiag_sb = big.tile([128, 128], BF16)
            nc.sync.dma_start(out=mdiag_sb, in_=mdiag[:, :])
            mfar_sb = big.tile([128, 128], BF16)
            nc.sync.dma_start(out=mfar_sb, in_=mfar[:, :])
            for i in range(3, NT):
                nc.sync.dma_start(out=xT_sb[:, i, :, :], in_=xTt[i])
            wp_sb = big.tile([128, 4, 2048], BF16)
            for g in range(2):
                nc.sync.dma_start(
                    out=wp_sb[:, 2 * g : 2 * g + 2, :],
                    in_=wp[256 * g : 256 * (g + 1), :].rearrange(
                        "(d p) n -> p d n", p=128
                    ),
                )

            ident = big.tile([128, 128], BF16)
            make_identity(nc, ident)

            # ---- persistent intermediates ----
            qT_sb = big.tile([128, 4, 2048], BF16)    # [d, h, t] normalized q
            kT_sb = big.tile([128, 2, 2048], BF16)    # [d, hk, t] normalized k
            v_sb = big.tile([128, NT, 2, 132], BF16)  # [t, i, hk, dv(+ones)]
            nc.vector.memset(v_sb[:, :, :, 128:129], 1.0)
            gates_sb = big.tile([128, NT, 6], FP32)   # [t, i, (gv0,gv1,ag0..3)]
            yT_sb = big.tile([128, 4, 2048], BF16)    # [dv, h, t]

            # preload the sigmoid act table during the initial DMA wait
            warm = small.tile([128, 1], FP32, tag="warm")
            nc.scalar.activation(warm, v_sb[:, 0, 0, 128:129], AF.Sigmoid)

            # ---- per-tile pipelined stages ----
            # A(i):  gates, q/kv projections, k_raw evac, v-assembly, q rope
            # B1(i): k-shift MMs, k rope, rmsnorm (runs while A(i+1) MMs)
            # B2(i): transposes -> qT/kT (after A(i+1) MMs; chain done)
            # PE stream: [ksh(i-1)] [gates(i) q(i) kv(i)] [tp(i-1)] ...
            def stage_a(i):
                zg_ps = pat.tile([128, 6], FP32, tag="att")
                nc.tensor.matmul(
                    zg_ps[:, 0:2], xT_sb[0:32, i, 0, :], wveg_sb,
                    start=True, stop=True,
                )
                nc.tensor.matmul(
                    zg_ps[:, 2:6], xT_sb[0:12, i, 0, :], wag_sb[0:12, :],
                    start=True, stop=True,
                )
                nc.scalar.activation(gates_sb[:, i, :], zg_ps, AF.Sigmoid)
                q_ps = ps.tile([128, 512], FP32, tag="proj")
                for cc in range(NCC):
                    nc.tensor.matmul(
                        q_ps, xT_sb[:, i, cc, :], wqkv_sb[:, cc, 0:512],
                        start=(cc == 0), stop=(cc == NCC - 1),
                    )
                kv_ps = ps.tile([128, 512], FP32, tag="proj")
                for cc in range(NCC):
                    nc.tensor.matmul(
                        kv_ps, xT_sb[:, i, cc, :], wqkv_sb[:, cc, 512:1024],
                        start=(cc == 0), stop=(cc == NCC - 1),
                    )
                # k natural (pre-shift); kwork bufs=3 keeps the previous
                # tile's buffer alive for the cross-tile shift row
                k_raw = kwork.tile([128, 2, 128], BF16, tag="kraw")
                nc.vector.tensor_copy(k_raw, kv_ps[:, 0:256].rearrange("p (h d) -> p h d", h=2))
                # q evacuation + rope (3 ops: swap-half trick, sign in sind)
                qr = work.tile([128, 4, 128], BF16, tag="qr")
                nc.any.tensor_copy(qr, q_ps.rearrange("p (h d) -> p h d", h=4))
                # v with ve gating: v_sb = (ve2 * gv) + v_ps (2*sigmoid in ve2)
                for hk in range(2):
                    nc.vector.scalar_tensor_tensor(
                        out=v_sb[:, i, hk, 0:128],
                        in0=ve_sb[:, i, bass.ts(hk, 128)],
                        scalar=gates_sb[:, i, hk : hk + 1],
                        in1=kv_ps[:, bass.ts(hk, 128)],
                        op0=ALU.mult,
                        op1=ALU.add,
                    )
                cb4 = bass.AP(cos_sb.tensor, cos_sb[:, i, :].offset,
                              [list(cos_sb[:, i, :].ap[0]), [0, 4], [1, 128]])
                sb4 = bass.AP(sin_sb.tensor, sin_sb[:, i, :].offset,
                              [list(sin_sb[:, i, :].ap[0]), [0, 4], [1, 128]])
                tc_ = work.tile([128, 4, 128], BF16, tag="tc")
                nc.vector.tensor_tensor(tc_, qr, cb4, op=ALU.mult)
                tsg = work.tile([128, 4, 128], BF16, tag="tsg")
                nc.vector.tensor_tensor(tsg, _swap_halves(qr), sb4, op=ALU.mult)
                qro = work.tile([128, 4, 128], BF16, tag="qro")
                nc.vector.tensor_tensor(qro, tc_, tsg, op=ALU.add)
                return k_raw, qro

            def stage_b1(i, k_raw, prev_kraw, qro):
                # key shift via PE: row t <- row t-1 (upper halves)
                ksh_ps = pat.tile([128, 2, 64], FP32, tag="att")
                nc.tensor.matmul(
                    ksh_ps, s1t_sb, k_raw[:, :, 64:128],
                    start=True, stop=False,
                )
                nc.tensor.matmul(
                    ksh_ps, s00t_sb if i == 0 else s0t_sb,
                    prev_kraw[:, :, 64:128],
                    start=False, stop=True,
                )
                ksh = work.tile([128, 2, 64], BF16, tag="ksh")
                nc.vector.tensor_copy(ksh, ksh_ps)
                # k rope (6 ops: halves live in different tensors)
                kro = work.tile([128, 2, 128], BF16, tag="kro")
                k1 = k_raw[:, :, 0:64]
                cb2l = bass.AP(cos_sb.tensor, cos_sb[:, i, :].offset,
                               [list(cos_sb[:, i, :].ap[0]), [0, 2], [1, 64]])
                sb2l = bass.AP(sin_sb.tensor, sin_sb[:, i, :].offset,
                               [list(sin_sb[:, i, :].ap[0]), [0, 2], [1, 64]])
                t1 = work.tile([128, 2, 64], BF16, tag="t1")
                t2 = work.tile([128, 2, 64], BF16, tag="t2")
                nc.vector.tensor_tensor(t1, k1, cb2l, op=ALU.mult)
                nc.vector.tensor_tensor(t2, ksh, sb2l, op=ALU.mult)
                nc.vector.tensor_tensor(kro[:, :, 0:64], t1, t2, op=ALU.add)
                nc.vector.tensor_tensor(t1, ksh, cb2l, op=ALU.mult)
                nc.vector.tensor_tensor(t2, k1, sb2l, op=ALU.mult)
                nc.vector.tensor_tensor(kro[:, :, 64:128], t1, t2, op=ALU.subtract)
                # rmsnorm: fused square+reduce, then fast-rsqrt on DVE
                ssq = small.tile([128, 6], FP32, tag="ssq")
                scr = work.tile([128, 128], BF16, tag="scr")
                for h in range(4):
                    nc.vector.tensor_tensor_reduce(
                        out=scr, in0=qro[:, h, :], in1=qro[:, h, :],
                        scale=1.0, scalar=128.0 * EPS,
                        op0=ALU.mult, op1=ALU.add,
                        accum_out=ssq[:, h : h + 1],
                    )
                for hk in range(2):
                    nc.vector.tensor_tensor_reduce(
                        out=scr, in0=kro[:, hk, :], in1=kro[:, hk, :],
                        scale=1.0 / 128.0, scalar=EPS,
                        op0=ALU.mult, op1=ALU.add,
                        accum_out=ssq[:, 4 + hk : 5 + hk],
                    )
                # rstd = rsqrt(ssq): bit-magic + one Newton step
                yb = small.tile([128, 6], I32, tag="yb")
                zi = ssq.bitcast(I32)
                nc.vector.tensor_scalar(yb, zi, 1, None, ALU.logical_shift_right)
                nc.vector.tensor_scalar(yb, yb, -1, MAGIC, ALU.mult, ALU.add)
                yf = yb.bitcast(FP32)
                aa = small.tile([128, 6], FP32, tag="aa")
                nc.vector.tensor_tensor(aa, yf, yf, op=ALU.mult)
                nc.vector.tensor_tensor(aa, ssq, aa, op=ALU.mult)
                nc.vector.tensor_scalar(aa, aa, -0.5, 1.5, ALU.mult, ALU.add)
                rstd = small.tile([128, 6], FP32, tag="rstd")
                nc.vector.tensor_tensor(rstd, yf, aa, op=ALU.mult)
                for h in range(4):
                    nc.vector.tensor_scalar_mul(qro[:, h, :], qro[:, h, :], rstd[:, h : h + 1])
                for hk in range(2):
                    nc.vector.tensor_scalar_mul(kro[:, hk, :], kro[:, hk, :], rstd[:, 4 + hk : 5 + hk])
                return kro

            def stage_b2(i, qro, kro):
                ts = bass.ts(i, 128)
                tp_ps = pst.tile([128, 6, 128], BF16, tag="st")
                for h in range(4):
                    nc.tensor.transpose(tp_ps[:, h, :], qro[:, h, :], ident)
                for hk in range(2):
                    nc.tensor.transpose(tp_ps[:, 4 + hk, :], kro[:, hk, :], ident)
                nc.any.tensor_copy(qT_sb[:, :, ts], tp_ps[:, 0:4, :])
                nc.any.tensor_copy(kT_sb[:, :, ts], tp_ps[:, 4:6, :])

            sa = {}
            for i in range(NT):
                if i >= 1:
                    k_raw, qro = sa[i - 1]
                    prev_kraw = sa[i - 2][0] if i >= 2 else k_raw
                    kro = stage_b1(i - 1, k_raw, prev_kraw, qro)
                sa[i] = stage_a(i)
                if i >= 1:
                    stage_b2(i - 1, sa[i - 1][1], kro)
            k_raw, qro = sa[NT - 1]
            kro = stage_b1(NT - 1, k_raw, sa[NT - 2][0], qro)
            stage_b2(NT - 1, qro, kro)

            # ---- attention ----
            for i in range(NT):
                ts = bass.ts(i, 128)
                js = list(range(max(0, i - 8), i + 1))
                L = len(js)
                chunks = [js[k : k + 3] for k in range(0, L, 3)]
                for h in range(4):
                    hk = h // 2
                    y_ps = pat.tile([128, 132], FP32, tag="att")
                    g = 0
                    for ch in chunks:
                        st_ps = pst.tile([128, 3, 128], FP32, tag="st")
                        for idx, j in enumerate(ch):
                            nc.tensor.matmul(
                                st_ps[:, idx, :],
                                kT_sb[:, hk, bass.ts(j, 128)],
                                qT_sb[:, h, ts],
                                start=True, stop=True,
                            )
                        ex = exw.tile([128, 3, 128], BF16, tag="ex")
                        nc.scalar.activation(ex[:, 0 : len(ch), :], st_ps[:, 0 : len(ch), :], AF.Exp)
                        if i >= 8 and ch[0] == js[0]:
                            nc.vector.tensor_tensor(ex[:, 0, :], ex[:, 0, :], mfar_sb, op=ALU.mult)
                        if ch[-1] == i:
                            nc.vector.tensor_tensor(
                                ex[:, len(ch) - 1, :], ex[:, len(ch) - 1, :], mdiag_sb, op=ALU.mult
                            )
                        for idx, j in enumerate(ch):
                            nc.tensor.matmul(
                                y_ps[:, 0:129],
                                ex[:, idx, :],
                                v_sb[:, j, hk, 0:129],
                                start=(g == 0), stop=(g == L - 1),
                            )
                            g += 1
                    # factor = ag / rowsum
                    rs = small.tile([128, 1], FP32, tag="rs")
                    nc.vector.reciprocal(rs, y_ps[:, 128:129])
                    fac = small.tile([128, 1], FP32, tag="fac")
                    nc.vector.tensor_tensor(fac, rs, gates_sb[:, i, 2 + h : 3 + h], op=ALU.mult)
                    yn = work.tile([128, 128], BF16, tag="yn")
                    nc.vector.tensor_scalar_mul(yn, y_ps[:, 0:128], fac)
                    tp_y = pat.tile([128, 132], BF16, tag="att")
                    nc.tensor.transpose(tp_y[:, 0:128], yn, ident)
                    nc.any.tensor_copy(yT_sb[:, h, ts], tp_y[:, 0:128])

            # ---- wproj ----
            for i in range(NT):
                ts = bass.ts(i, 128)
                o_sb = work.tile([128, 2048], BF16, tag="osb")
                for c in range(4):
                    o_ps = ps.tile([128, 512], FP32, tag="proj")
                    for dc in range(4):
                        nc.tensor.matmul(
                            o_ps,
                            yT_sb[:, dc, ts],
                            wp_sb[:, dc, bass.ts(c, 512)],
                            start=(dc == 0), stop=(dc == 3),
                        )
                    nc.any.tensor_copy(o_sb[:, bass.ts(c, 512)], o_ps)
                nc.sync.dma_start(out=out[ts, :], in_=o_sb)
    nc.compile()
    return nc


def _get_nc():
    if "nc" not in _CACHE:
        _CACHE["nc"] = _build_nc()
    return _CACHE["nc"]


def kernel(**inputs):
    x = np.asarray(inputs["x"], np.float32)
    ve = np.asarray(inputs["ve"], np.float32)
    cos = np.asarray(inputs["cos"], np.float32).reshape(T, 64)
    sin = np.asarray(inputs["sin"], np.float32).reshape(T, 64)
    wq = np.asarray(inputs["wq"], np.float32)
    wk = np.asarray(inputs["wk"], np.float32)
    wv = np.asarray(inputs["wv"], np.float32)
    wproj = np.asarray(inputs["wproj"], np.float32)
    wveg = np.asarray(inputs["w_ve_gate"], np.float32)
    wag = np.asarray(inputs["w_attn_gate"], np.float32)
    proj_scalar = np.asarray(inputs["proj_scalar"], np.float32)

    ii, jj = np.meshgrid(np.arange(128), np.arange(128), indexing="ij")
    mdiag = (jj >= ii).astype(bf16)   # [k, q]: allowed q >= k
    mfar = (jj <= ii).astype(bf16)    # [k, q]: allowed q <= k
    s1t = np.eye(128, k=1).astype(bf16)           # out[t] = in[t-1], t>=1
    s0t = np.zeros((128, 128), np.float32)
    s0t[127, 0] = 1.0                              # out[0] = prev tile row 127
    s0t = s0t.astype(bf16)
    s00t = np.zeros((128, 128), np.float32)
    s00t[0, 0] = 1.0                               # tile 0: out[0] = own row 0
    s00t = s00t.astype(bf16)

    cosdup = np.concatenate([cos, cos], axis=1)    # [T, 128]
    sinsg = np.concatenate([sin, -sin], axis=1)    # sign-folded for swap rope
    cosd = np.ascontiguousarray(
        cosdup.reshape(NT, 128, 128).transpose(1, 0, 2)).astype(bf16)
    sind = np.ascontiguousarray(
        sinsg.reshape(NT, 128, 128).transpose(1, 0, 2)).astype(bf16)

    in_maps = []
    for core in range(8):
        b, tp = core // 4, core % 4
        # xTt[i, p, cc, tt] = x[b][i*128+tt, cc*128+p]
        xTt = np.ascontiguousarray(
            x[b].reshape(NT, 128, NCC, 128).transpose(0, 3, 2, 1)).astype(bf16)
        in_maps.append({
            "xTt": xTt,
            "ve2": (2.0 * ve[b][:, tp * 256 : (tp + 1) * 256]).astype(bf16),
            "wqkv": np.ascontiguousarray(np.concatenate([
                wq[:, tp * 512 : (tp + 1) * 512],
                wk[:, tp * 256 : (tp + 1) * 256],
                wv[:, tp * 256 : (tp + 1) * 256]], axis=1)).astype(bf16),
            "wp": np.ascontiguousarray(wproj[tp * 512 : (tp + 1) * 512, :]).astype(bf16),
            "wveg": np.ascontiguousarray(wveg[:, 2 * tp : 2 * tp + 2]).astype(bf16),
            "wag": np.ascontiguousarray(wag[:, 4 * tp : 4 * tp + 4]).astype(bf16),
            "cosd": cosd, "sind": sind, "mdiag": mdiag, "mfar": mfar,
            "s1t": s1t, "s0t": s0t, "s00t": s00t,
        })

    res = run_bass_kernel_spmd(_get_nc(), in_maps, core_ids=list(range(8)))
    _CACHE["last_res"] = res
    out = np.zeros((2, T, 2048), np.float32)
    for core in range(8):
        b = core // 4
        out[b] += res.results[core]["out"].astype(np.float32)
    out *= (1.0 + proj_scalar[0])
    return out
